# revision 44
# baseline (speedup 1.0000x reference)
"""FEDformer encoder layer on 8 TRN2 NeuronCores — batch-data-parallel Bass kernel.

Strategy (self-contained; shapes hardcoded):
  B=16,L=2048,D=512,H=8,E=64,M=64,DFF=2048; 8 cores x 2 batches each; no collectives.

  Math restructuring (validated against the jax reference):
   - rfft+mode-gather == x @ Fcat where Fcat[l, 0:64]=cos(2*pi*k_j*l/L),
     Fcat[l, 64:128]=-sin(...), k_j = mode_index.
   - Wq and Wo are applied in mode space (16x cheaper); k/v projections are
     dead code in the reference.
   - irfft of a spectrum with only bins 0..63 populated == P @ C2S2.
   - The Fourier branch contributes ~1e-4 absolute to an O(1) output, so the
     whole mode path (DFT input, DFT matrix, mode weights, mode data) runs in
     fp8e4m3; per-element ~6% error on a 1e-4 contribution is negligible.
   - series-decomp: the two moving averages are banded Toeplitz matmuls on the
     (otherwise idle) tensor engine, in token-major layout reached via the DMA
     XBAR transpose (SBUF->SBUF, bf16).  Output chunks of 96 tokens come from
     overlapping 128-token input windows (stride 96), so one stationary matrix
     per window size serves all interior chunks and replicate-padding folds
     into dedicated first/last-chunk stationaries.  The K=2 softmax gate is a
     sigmoid of weight/bias deltas; combines run token-major on Vector reading
     the PSUM means directly; the smooth mean M transposes back (bf16) and the
     final residual r = u - M keeps the carried stream in fp32.
   - FFN entirely in fp8e4 with DoubleRow matmuls; host prescales W1/W2.  The
     psum drain is a single vector scalar_tensor_tensor reading PSUM and
     adding the fp32 residual in place (no scalar copy / gpsimd add).
   - x reaches the device once as bf16 (x + bo folded on host): the iDFT
     residual add constructs the fp32 stream u = psy + xtb directly.

  Layout: device works feature-major ([D, L]) for the residual/FFN stream and
  token-major ([L-chunk, D]) for the decomposition means.
"""

import numpy as np

B, L, D, H, M, DFF = 16, 2048, 512, 8, 64, 2048
E = D // H
NC_ = 8
BLOC = B // NC_          # batches per core
NDC = D // 128           # 4 feature tiles
NFF = DFF // 128         # 16 dff tiles
NLC = L // 128           # 16 token chunks of 128
NTC = L // 512           # 4 token chunks of 512

# decomposition chunking: 16 non-overlapping 128-token windows; each mean is
# up to 3 accumulating matmuls (prev-tail band, in-window band, next-head
# band) so every engine access sits at partition offset 0 (the HW rejects
# wide accesses at nonzero partition offsets).
NW = 16

_prog_cache = {}
_fixn = [0]


def _fix_sync_waits(nc, max_waits=1, max_updates=4):
    """Split >max sem-waits/updates per instruction onto adjacent nops.

    The AWS neuronx-cc walrus rejects instructions carrying too many sync
    commands; Tile's tail drain aggregates one wait per outstanding semaphore.
    Engine-order execution makes the split semantically identical.
    """
    import concourse.mybir as mybir

    for f in nc.m.functions:
        for bb in f.blocks:
            insts = bb.instructions
            i = 0
            while i < len(insts):
                ins = insts[i]
                si = ins.sync_info
                if si is not None and si.on_wait and len(si.on_wait) > max_waits:
                    waits = list(si.on_wait)
                    si.on_wait = waits[-max_waits:]
                    rest = waits[:-max_waits]
                    chunks = [rest[j:j + max_waits]
                              for j in range(0, len(rest), max_waits)]
                    for c in reversed(chunks):
                        _fixn[0] += 1
                        nop = mybir.InstNoOp(name=f"I-fixw-{_fixn[0]}", ins=[], outs=[])
                        nop.engine = ins.engine
                        nop.sync_info = mybir.SyncInfo(on_wait=c, on_update=[])
                        insts.insert(i, nop)
                        i += 1
                if si is not None and si.on_update and len(si.on_update) > max_updates:
                    ups = list(si.on_update)
                    si.on_update = ups[:max_updates]
                    rest = ups[max_updates:]
                    chunks = [rest[j:j + max_updates]
                              for j in range(0, len(rest), max_updates)]
                    for c in chunks:
                        _fixn[0] += 1
                        nop = mybir.InstNoOp(name=f"I-fixu-{_fixn[0]}", ins=[], outs=[])
                        nop.engine = ins.engine
                        nop.sync_info = mybir.SyncInfo(on_wait=[], on_update=c)
                        insts.insert(i + 1, nop)
                        i += 1
                i += 1


def _build_program(need_bq, j0, w2scale, w1scale, w8scale, fix=True):
    import concourse.bass as bass
    import concourse.mybir as mybir
    from concourse.tile import TileContext

    F32 = mybir.dt.float32
    BF16 = mybir.dt.bfloat16
    FP8 = mybir.dt.float8e4
    AF = mybir.ActivationFunctionType
    OP = mybir.AluOpType

    nc = bass.Bass()

    # ---- DRAM I/O ----
    XTB = nc.dram_tensor("XTB", [BLOC, D, L], BF16, kind="ExternalInput")
    XBF = nc.dram_tensor("XBF", [BLOC, 128, NLC * D], FP8, kind="ExternalInput")
    FCT = nc.dram_tensor("FCT", [128, NLC * 128], FP8, kind="ExternalInput")
    C2S2 = nc.dram_tensor("C2S2", [128, L], BF16, kind="ExternalInput")
    WQT = nc.dram_tensor("WQT", [D, D], BF16, kind="ExternalInput")
    WOT = nc.dram_tensor("WOT", [D, D], BF16, kind="ExternalInput")
    WPK = nc.dram_tensor("WPK", [H, 128, M * 128], FP8, kind="ExternalInput")
    W1T = nc.dram_tensor("W1T", [D, DFF], FP8, kind="ExternalInput")
    W2T = nc.dram_tensor("W2T", [DFF, D], FP8, kind="ExternalInput")
    EYE = nc.dram_tensor("EYE", [128, 128], BF16, kind="ExternalInput")
    BQ4 = nc.dram_tensor("BQ4", [128, NDC], F32, kind="ExternalInput")
    AMAT = nc.dram_tensor("AMAT", [128, 1280], BF16, kind="ExternalInput")
    DECS = nc.dram_tensor("DECS", [128, 4], F32, kind="ExternalInput")
    OUT_T = nc.dram_tensor("OUT_T", [BLOC, D, L], F32, kind="ExternalOutput")

    with TileContext(nc) as tc:
        # ---------- persistent pools (explicit LIFO close at the end) ------
        cst = tc.tile_pool(name="cst", bufs=1)
        cstp = cst.__enter__()
        dec = tc.tile_pool(name="dec", bufs=1)
        decp = dec.__enter__()
        dtok = tc.tile_pool(name="dtok", bufs=4)
        dtokp = dtok.__enter__()

        # DMA issue order: what the front needs first.
        fct = cstp.tile([128, NLC * 128], FP8, name="fct")
        nc.sync.dma_start(out=fct[:], in_=FCT[:])
        wqt = [cstp.tile([128, D], BF16, name=f"wqt{i}") for i in range(NDC)]
        wot = [cstp.tile([128, D], BF16, name=f"wot{i}") for i in range(NDC)]
        eye = cstp.tile([128, 128], BF16, name="eye")
        c2s2 = cstp.tile([128, L], BF16, name="c2s2")
        amat = cstp.tile([128, 1280], BF16, name="amat")
        decs = cstp.tile([128, 4], F32, name="decs")
        bq4 = None

        mt = [[cstp.tile([128, L], F32, name=f"m_{b}_{dc}") for dc in range(NDC)]
              for b in range(BLOC)]
        r18 = [[cstp.tile([128, 2, L], FP8, name=f"r18_{b}_{kp}") for kp in range(2)]
               for b in range(BLOC)]
        # decomposition working tiles (persistent; single-buffered)
        ubf2 = decp.tile([128, NDC, L], BF16, name="ubf2")
        ut2 = decp.tile([128, NDC, NW, 128], BF16, name="ut2")
        mta2 = decp.tile([128, NW, D], BF16, name="mta2")
        mfm2 = decp.tile([128, NW, NDC, 128], BF16, name="mfm2")

        # ---------- series decomposition via tensor-engine banded means ----
        def decomp_pe(b, msp, dw_col, db_col, want_r1):
            """mt[b][*] (fp32 [128, L]) -> series-decomp residual, in place.

            u -> bf16 copy -> one batched DMA-xbar blocked transpose into
            token-major 128-token windows -> m25 / (m13-m25) as banded
            matmuls -> token-major combines on V reading PSUM -> smooth mean
            transposed back (4-chunk groups) -> r = u - M on gpsimd (fp32
            stream untouched by the bf16 mean path).
            """
            # cast + transpose per half-L: the half-0 pieces only depend on
            # the first two t4 slices of mt (u-adds / FFN drains), so the
            # early mean chunks start while the producer is still finishing
            for hf in range(2):
                for dc in range(NDC):
                    hs = slice(hf * 1024, (hf + 1) * 1024)
                    nc.scalar.activation(ubf2[:, dc, hs], mt[b][dc][:, hs],
                                         AF.Copy)
                    nc.sync.dma_start(out=ut2[:, dc, 8 * hf:8 * hf + 8, :],
                                      in_=ubf2[:, dc, hs], transpose=True)
            for c in range(NW):
                v = 1 if c == 0 else (3 if c == NW - 1 else 2)
                seq = ([(0, c - 1)] if c > 0 else []) + [(v, c)] + \
                    ([(4, c + 1)] if c < NW - 1 else [])
                m25 = msp.tile([128, D], F32, name="m25", tag="m25")
                dlt = msp.tile([128, D], F32, name="dlt", tag="dlt")
                # each PSUM column region must finish its accumulation group
                # before the next region starts (interleaved groups corrupt);
                # dc-pairs share one matmul via a 3D strided rhs (free 256)
                for ps, boff in ((m25, 0), (dlt, 5)):
                    for dp in range(2):
                        for i, (blk, wc) in enumerate(seq):
                            nc.tensor.matmul(
                                ps[:, dp * 256:(dp + 1) * 256],
                                amat[:, (boff + blk) * 128:(boff + blk + 1) * 128],
                                ut2[:, 2 * dp:2 * dp + 2, wc, :],
                                start=(i == 0), stop=(i == len(seq) - 1))
                g = dtokp.tile([128, NDC, 128], BF16, name="g", tag="g")
                nc.scalar.activation(g[:], ut2[:, :, c, :], AF.Sigmoid,
                                     scale=decs[:, dw_col:dw_col + 1],
                                     bias=decs[:, db_col:db_col + 1])
                q = dtokp.tile([128, D], BF16, name="q", tag="q")
                nc.vector.tensor_tensor(q[:], g.rearrange("p a b -> p (a b)"),
                                        dlt[:], OP.mult)
                nc.vector.tensor_tensor(mta2[:, c, :], m25[:], q[:], OP.add)
            for gi in range(4):
                nc.sync.dma_start(
                    out=mfm2[:, 4 * gi:4 * gi + 4, :, :],
                    in_=mta2[:, 4 * gi:4 * gi + 4, :], transpose=True)
            # r = u - M, split per half-L and across V/G so the tail
            # pipelines with the group transposes above
            for dc in range(NDC):
                mtv = mt[b][dc].rearrange("p (c t) -> p c t", t=128)
                for hf in range(2):
                    sl = (slice(None), slice(8 * hf, 8 * hf + 8), slice(None))
                    eng = nc.vector if (dc + hf) % 2 == 0 else nc.gpsimd
                    eng.tensor_tensor(mtv[sl], mtv[sl],
                                      mfm2[:, 8 * hf:8 * hf + 8, dc, :],
                                      OP.subtract)
                    if not want_r1:
                        nc.sync.dma_start(
                            out=OUT_T[b, dc * 128:(dc + 1) * 128,
                                      hf * 1024:(hf + 1) * 1024],
                            in_=mt[b][dc][:, hf * 1024:(hf + 1) * 1024])
                if want_r1:
                    nc.scalar.activation(r18[b][dc // 2][:, dc % 2, :],
                                         mt[b][dc][:], AF.Copy)

        # ---------- FFN weights (issued early; used after the Fourier phase)
        ffnw = tc.tile_pool(name="ffnw", bufs=1)
        ffnwp = ffnw.__enter__()
        w1dr = [ffnwp.tile([128, 2, DFF], FP8, name=f"w1dr{i}")
                for i in range(2)]
        w2dr = [ffnwp.tile([128, 2, D], FP8, name=f"w2dr{i}")
                for i in range(NFF // 2)]

        # ---------- Fourier branch (fp8 mode path) ----------
        ph1s = tc.tile_pool(name="ph1s", bufs=1)
        ph1sp = ph1s.__enter__()
        pcat = [ph1sp.tile([128, D], BF16, name=f"pcat{b}") for b in range(BLOC)]

        msp_cm = tc.tile_pool(name="msp", bufs=2, space="PSUM")

        with tc.tile_pool(name="ph1a", bufs=2, space="PSUM") as frp, \
             tc.tile_pool(name="ph1t", bufs=1, space="PSUM") as frpt, \
             tc.tile_pool(name="wpkp", bufs=4) as wpkp, \
             tc.tile_pool(name="xbfp", bufs=2) as xbfp:
            # x token-major fp8, streamed in quarter-L chunks
            xbfq = {}
            for b in range(BLOC):
                for qc in range(4):
                    xbfq[(b, qc)] = xbfp.tile([128, 4 * D], FP8,
                                              name=f"xb{b}_{qc}", tag="xb")
            for b in range(BLOC):
                for qc in range(4):
                    nc.sync.dma_start(out=xbfq[(b, qc)][:],
                                      in_=XBF[b][:, qc * 4 * D:(qc + 1) * 4 * D])
            # mode-mix weight stream: all quarters issued up front; the
            # 4-buffer pool self-clocks arrival against consumption
            wpk_tiles = []
            for h in range(H):
                for q in range(4):
                    wq = wpkp.tile([128, 16 * 128], FP8, name=f"wpk{h}_{q}",
                                   tag="wpk")
                    nc.sync.dma_start(out=wq[:],
                                      in_=WPK[h][:, q * 2048:(q + 1) * 2048])
                    wpk_tiles.append(wq)
            for i in range(NDC):
                nc.sync.dma_start(out=wqt[i][:], in_=WQT[i * 128:(i + 1) * 128, :])
            nc.sync.dma_start(out=eye[:], in_=EYE[:])
            nc.sync.dma_start(out=c2s2[:], in_=C2S2[:])
            for i in range(NDC):
                nc.sync.dma_start(out=wot[i][:], in_=WOT[i * 128:(i + 1) * 128, :])
            nc.sync.dma_start(out=decs[:], in_=DECS[:])
            nc.sync.dma_start(out=amat[:], in_=AMAT[:])
            if need_bq:
                bq4 = cstp.tile([128, NDC], F32, name="bq4")
                nc.sync.dma_start(out=bq4[:], in_=BQ4[:])

            qt = [[None] * NDC for _ in range(BLOC)]
            for b in range(BLOC):
                # DFT: psD[m-ext, d] = sum_lc fct_lc^T @ xbf_lc (16 matmuls)
                psd = frp.tile([128, D], F32, name="psD", tag="psD")
                for lc in range(NLC):
                    nc.tensor.matmul(
                        psd[:], fct[:, lc * 128:(lc + 1) * 128],
                        xbfq[(b, lc // 4)][:, (lc % 4) * D:(lc % 4 + 1) * D],
                        start=(lc == 0), stop=(lc == NLC - 1))
                xsn = ph1sp.tile([128, D], BF16, name=f"xsn{b}", tag="xsn")
                nc.scalar.copy(xsn[:], psd[:])
                # transpose to xselT[d, m-ext]
                xselT = ph1sp.tile([128, NDC * 128], BF16, name=f"xselT{b}",
                                   tag="xselT")
                for dc in range(NDC):
                    pst = frpt.tile([128, 128], BF16, name="psT", tag="psT")
                    nc.tensor.transpose(pst[:], xsn[:, dc * 128:(dc + 1) * 128],
                                        eye[:])
                    nc.scalar.copy(xselT[:, dc * 128:(dc + 1) * 128], pst[:])
                # q-projection in mode space: QT[dout, m-ext]
                for do in range(NDC):
                    qt[b][do] = ph1sp.tile([128, 128], BF16, name=f"qt{b}_{do}",
                                           tag=f"qt{b}_{do}")
                    ps = frp.tile([128, 128], F32, name="psQ", tag="psQ")
                    for dc in range(NDC):
                        nc.tensor.matmul(
                            ps[:], wqt[dc][:, do * 128:(do + 1) * 128],
                            xselT[:, dc * 128:(dc + 1) * 128],
                            start=(dc == 0), stop=(dc == NDC - 1))
                    if need_bq:
                        nc.vector.tensor_tensor(
                            ps[:, j0:j0 + 1], ps[:, j0:j0 + 1],
                            bq4[:, do:do + 1], OP.add)
                    nc.scalar.copy(qt[b][do][:], ps[:])

            # mode mix: per head, per mode, complex ExE channel mix (fp8,
            # no DoubleRow: full-column bf16/fp8 weights get FWL).
            # RH_h rows: 0:64 = Qre e-rows, 64:128 = Qim e-rows; col = 2m + b
            rh = [ph1sp.tile([128, 128], FP8, name=f"rh{h}", tag=f"rh{h}")
                  for h in range(H)]
            for h in range(H):
                src_do, r0 = h // 2, (h % 2) * 64
                for b in range(BLOC):
                    rhv = rh[h].rearrange("p (m t) -> p m t", t=2)
                    nc.scalar.copy(rhv[0:64, :, b], qt[b][src_do][r0:r0 + 64, 0:64])
                    nc.scalar.copy(rhv[64:128, :, b], qt[b][src_do][r0:r0 + 64, 64:128])
            otre = [[ph1sp.tile([128, M], BF16, name=f"otre{b}_{dc}",
                                tag=f"otre{b}{dc}")
                     for dc in range(NDC)] for b in range(BLOC)]
            otim = [[ph1sp.tile([128, M], BF16, name=f"otim{b}_{dc}",
                                tag=f"otim{b}{dc}")
                     for dc in range(NDC)] for b in range(BLOC)]
            for h in range(H):
                psm = frp.tile([128, 128], F32, name="psM", tag="psM")
                for q in range(4):
                    wpk_q = wpk_tiles[h * 4 + q]
                    for mq in range(16):
                        m = q * 16 + mq
                        nc.tensor.matmul(
                            psm[:, 2 * m:2 * m + 2],
                            wpk_q[:, mq * 128:(mq + 1) * 128],
                            rh[h][:, 2 * m:2 * m + 2],
                            start=True, stop=True)
                psv = psm.rearrange("p (m t) -> p m t", t=2)
                dc, r0 = h // 2, (h % 2) * 64
                for b in range(BLOC):
                    nc.scalar.activation(otre[b][dc][r0:r0 + 64, :],
                                         psv[0:64, :, b], AF.Copy,
                                         scale=1.0 / w8scale)
                    nc.scalar.activation(otim[b][dc][r0:r0 + 64, :],
                                         psv[64:128, :, b], AF.Copy,
                                         scale=1.0 / w8scale)

            # bulk loads queued behind the phase-1-critical WPK stream:
            # x (feature-major bf16, +bo folded) lands in the idle decomp
            # buffers (ubf2 for b0, mta2 for b1), then the FFN weights
            mta2f = mta2.rearrange("p a b -> p (a b)")
            for dc in range(NDC):
                nc.sync.dma_start(out=ubf2[:, dc, :],
                                  in_=XTB[0, dc * 128:(dc + 1) * 128, :])
            for dc in range(NDC):
                nc.sync.dma_start(out=mta2f[:, dc * L:(dc + 1) * L],
                                  in_=XTB[1, dc * 128:(dc + 1) * 128, :])
            for i in range(2):
                nc.sync.dma_start(out=w1dr[i][:, 0, :],
                                  in_=W1T[(2 * i) * 128:(2 * i + 1) * 128, :])
                nc.sync.dma_start(out=w1dr[i][:, 1, :],
                                  in_=W1T[(2 * i + 1) * 128:(2 * i + 2) * 128, :])
            for i in range(NFF // 2):
                nc.sync.dma_start(out=w2dr[i][:, 0, :],
                                  in_=W2T[(2 * i) * 128:(2 * i + 1) * 128, :])
                nc.sync.dma_start(out=w2dr[i][:, 1, :],
                                  in_=W2T[(2 * i + 1) * 128:(2 * i + 2) * 128, :])

        # ph1a PSUM closed; open the means PSUM pool (lives through the rest)
        msp = msp_cm.__enter__()

        with tc.tile_pool(name="pswo", bufs=1, space="PSUM") as pswo, \
             tc.tile_pool(name="psy", bufs=2, space="PSUM") as psyp:
            for b in range(BLOC):
                # Wo projection, transposed orientation: pcat[m-ext, dout]
                for ro, ot in ((0, otre[b]), (64, otim[b])):
                    psw = pswo.tile([M, D], F32, name="psW", tag="psW")
                    for dc in range(NDC):
                        nc.tensor.matmul(
                            psw[:], ot[dc][:], wot[dc][:],
                            start=(dc == 0), stop=(dc == NDC - 1))
                    nc.scalar.copy(pcat[b][ro:ro + 64, :], psw[:])
                # iDFT + u = x+bo + yW   (feature-major, fp32, writes mt)
                for dc in range(NDC):
                    for t4 in range(NTC):
                        psy = psyp.tile([128, 512], F32, name="psY", tag="psY")
                        nc.tensor.matmul(
                            psy[:], pcat[b][:, dc * 128:(dc + 1) * 128],
                            c2s2[:, t4 * 512:(t4 + 1) * 512],
                            start=True, stop=True)
                        if b == 0:
                            xsl = ubf2[:, dc, t4 * 512:(t4 + 1) * 512]
                        else:
                            xsl = mta2.rearrange("p a b -> p (a b)")[
                                :, dc * L + t4 * 512:dc * L + (t4 + 1) * 512]
                        nc.vector.tensor_tensor(
                            mt[b][dc][:, t4 * 512:(t4 + 1) * 512],
                            psy[:], xsl, OP.add)
            # issued after BOTH batches' u-adds: the decomp scratch buffers
            # double as the landing zone for x (ubf2: b0, mta2: b1), so the
            # b1 reads must precede every decomp write in program order
            decomp_pe(0, msp, 0, 1, True)

        ph1s.__exit__(None, None, None)

        # ---------- FFN (fp8e4 DoubleRow; host-prescaled W1/W2) ----------
        def ffn(b):
            for t4 in range(NTC):
                t0, t1 = t4 * 512, (t4 + 1) * 512
                gq2 = [gqp.tile([128, 2, 512], FP8, name=f"gq{fp}",
                                tag=f"gq{fp}") for fp in range(NFF // 2)]
                for fp in range(NFF // 2):
                    for k in range(2):
                        ff = 2 * fp + k
                        psh = pshp.tile([128, 512], F32, name="psH", tag="psH")
                        for kp in range(2):
                            nc.tensor.matmul(
                                psh[:],
                                w1dr[kp][:, :, ff * 128:(ff + 1) * 128],
                                r18[b][kp][:, :, t0:t1],
                                start=(kp == 0), stop=(kp == 1),
                                perf_mode=mybir.MatmulPerfMode.DoubleRow)
                        nc.scalar.activation(gq2[fp][:, k, :], psh[:], AF.Gelu,
                                             scale=1.0 / w1scale)
                for do in range(NDC):
                    psf = psfp.tile([128, 512], F32, name="psF", tag="psF")
                    for fp in range(NFF // 2):
                        nc.tensor.matmul(
                            psf[:],
                            w2dr[fp][:, :, do * 128:(do + 1) * 128],
                            gq2[fp][:, :, :],
                            start=(fp == 0), stop=(fp == NFF // 2 - 1),
                            perf_mode=mybir.MatmulPerfMode.DoubleRow)
                    # drain + residual add in one vector op reading PSUM
                    sl = mt[b][do][:, t0:t1]
                    nc.vector.scalar_tensor_tensor(
                        sl, psf[:], 1.0 / w2scale, sl, OP.mult, OP.add)

        with tc.tile_pool(name="gqp", bufs=1) as gqp, \
             tc.tile_pool(name="pshp", bufs=2, space="PSUM") as pshp, \
             tc.tile_pool(name="psfp", bufs=2, space="PSUM") as psfp:
            decomp_pe(1, msp, 0, 1, True)
            ffn(0)
            decomp_pe(0, msp, 2, 3, False)
            ffn(1)
            decomp_pe(1, msp, 2, 3, False)

        ffnw.__exit__(None, None, None)
        msp_cm.__exit__(None, None, None)
        dtok.__exit__(None, None, None)
        dec.__exit__(None, None, None)
        cst.__exit__(None, None, None)

    if fix:
        _fix_sync_waits(nc)
    return nc


def _build_amat():
    """[128, 1280] float64: 10 [128,128] blocks
    [A25_m | A25_first | A25_int | A25_last | A25_p | B_m | B_first | B_int |
    B_last | B_p].  mean(c) = A_m^T u[c-1] + A_0^T u[c] + A_p^T u[c+1] with
    replicate padding folded into the first/last in-window variants.
    B = A13 - A25 so the delta matmul yields m13 - m25 directly."""
    def build(k):
        p = (k - 1) // 2
        A_m = np.zeros((128, 128))
        A_p = np.zeros((128, 128))
        A_f = np.zeros((128, 128))
        A_i = np.zeros((128, 128))
        A_l = np.zeros((128, 128))
        for j in range(128):
            for d in range(-p, p + 1):
                t = j + d
                if t < 0:
                    A_m[128 + t, j] += 1.0 / k
                    A_f[0, j] += 1.0 / k
                elif t > 127:
                    A_p[t - 128, j] += 1.0 / k
                    A_l[127, j] += 1.0 / k
                if 0 <= t <= 127:
                    A_f[t, j] += 1.0 / k
                    A_i[t, j] += 1.0 / k
                    A_l[t, j] += 1.0 / k
        return [A_m, A_f, A_i, A_l, A_p]
    A13 = build(13)
    A25 = build(25)
    return np.concatenate(A25 + [a13 - a25 for a13, a25 in zip(A13, A25)],
                          axis=1)


def _host_prep(inputs):
    import ml_dtypes
    bf16 = ml_dtypes.bfloat16
    fp8 = ml_dtypes.float8_e4m3
    x = np.asarray(inputs["x"], np.float32)
    w2t_pre = np.asarray(inputs["conv2_w"], np.float32).T
    w2scale = float(2.0 ** np.floor(np.log2(224.0 / np.abs(w2t_pre).max())))
    w1t_pre = np.asarray(inputs["conv1_w"], np.float32).T
    w1scale = float(2.0 ** np.floor(np.log2(224.0 / np.abs(w1t_pre).max())))
    modes = np.asarray(inputs["mode_index"]).astype(np.int64)
    l = np.arange(L, dtype=np.float64)
    ang = 2.0 * np.pi * np.outer(l, modes.astype(np.float64)) / L
    FC = np.concatenate([np.cos(ang), -np.sin(ang)], axis=1)          # [L, 128]
    m_out = np.arange(M, dtype=np.float64)
    w = np.where(m_out == 0, 1.0, 2.0) / L
    ang2 = 2.0 * np.pi * np.outer(m_out, l) / L
    C2 = np.concatenate([w[:, None] * np.cos(ang2),
                         w[:, None] * -np.sin(ang2)], axis=0)         # [128, L]

    FCT = FC.reshape(NLC, 128, 128).transpose(1, 0, 2).reshape(128, NLC * 128)

    wr = np.asarray(inputs["four_wr"], np.float64)   # [H, E, O, M]
    wi = np.asarray(inputs["four_wi"], np.float64)
    wpk = np.zeros((H, M, 128, 128), np.float64)
    wpk[:, :, 0:64, 0:64] = wr.transpose(0, 3, 1, 2)
    wpk[:, :, 0:64, 64:128] = wi.transpose(0, 3, 1, 2)
    wpk[:, :, 64:128, 0:64] = -wi.transpose(0, 3, 1, 2)
    wpk[:, :, 64:128, 64:128] = wr.transpose(0, 3, 1, 2)
    wmax = np.abs(wpk).max()
    w8scale = float(2.0 ** np.floor(np.log2(224.0 / max(wmax, 1e-30))))
    WPKh = (wpk * w8scale).transpose(0, 2, 1, 3).reshape(H, 128, M * 128)

    dec1_w = np.asarray(inputs["dec1_w"], np.float64)
    dec1_b = np.asarray(inputs["dec1_b"], np.float64)
    dec2_w = np.asarray(inputs["dec2_w"], np.float64)
    dec2_b = np.asarray(inputs["dec2_b"], np.float64)
    decs = np.zeros((128, 4), np.float32)
    decs[:, 0] = dec1_w[0] - dec1_w[1]
    decs[:, 1] = dec1_b[0] - dec1_b[1]
    decs[:, 2] = dec2_w[0] - dec2_w[1]
    decs[:, 3] = dec2_b[0] - dec2_b[1]

    bo = np.asarray(inputs["bo"], np.float32)
    bq = np.asarray(inputs["bq"], np.float32)
    zero_pos = np.nonzero(modes == 0)[0]
    need_bq = bool(len(zero_pos)) and bool(np.any(bq != 0))
    j0 = int(zero_pos[0]) if need_bq else 0
    BQ4 = np.ascontiguousarray((L * bq).reshape(NDC, 128).T).astype(np.float32)

    shared = {
        "FCT": FCT.astype(fp8),
        "C2S2": C2.astype(bf16),
        "WQT": np.ascontiguousarray(np.asarray(inputs["Wq"], np.float32).T).astype(bf16),
        "WOT": np.ascontiguousarray(np.asarray(inputs["Wo"], np.float32).T).astype(bf16),
        "WPK": WPKh.astype(fp8),
        "W1T": np.ascontiguousarray(w1t_pre * w1scale).astype(fp8),
        "W2T": np.ascontiguousarray(w2t_pre * w2scale).astype(fp8),
        "EYE": np.eye(128, dtype=np.float32).astype(bf16),
        "BQ4": BQ4,
        "AMAT": _build_amat().astype(bf16),
        "DECS": decs,
    }
    in_maps = []
    for c in range(NC_):
        xl = x[c * BLOC:(c + 1) * BLOC]                       # [2, L, D]
        xpb = xl + bo[None, None, :]
        XTBc = np.ascontiguousarray(xpb.transpose(0, 2, 1)).astype(bf16)
        xbf = xl.astype(fp8)                                  # [2, L, D]
        XBFc = np.ascontiguousarray(
            xbf.reshape(BLOC, NLC, 128, D).transpose(0, 2, 1, 3)
        ).reshape(BLOC, 128, NLC * D)
        im = dict(shared)
        im["XTB"] = XTBc
        im["XBF"] = XBFc
        in_maps.append(im)
    return in_maps, need_bq, j0, w2scale, w1scale, w8scale


def kernel(**inputs):
    from concourse.bass_utils import run_bass_kernel_spmd

    in_maps, need_bq, j0, w2scale, w1scale, w8scale = _host_prep(inputs)
    key = (need_bq, j0, w2scale, w1scale, w8scale)
    if key not in _prog_cache:
        _prog_cache[key] = _build_program(need_bq, j0, w2scale, w1scale, w8scale)
    nc = _prog_cache[key]
    res = run_bass_kernel_spmd(nc, in_maps, core_ids=list(range(NC_)))
    outs = []
    for c in range(NC_):
        ot = np.asarray(res.results[c]["OUT_T"])              # [2, D, L]
        outs.append(np.ascontiguousarray(ot.transpose(0, 2, 1)))
    return np.concatenate(outs, axis=0).astype(np.float32)


# revision 47
# speedup vs baseline: 1.0333x; 1.0333x over previous
"""FEDformer encoder layer on 8 TRN2 NeuronCores — batch-data-parallel Bass kernel.

Strategy (self-contained; shapes hardcoded):
  B=16,L=2048,D=512,H=8,E=64,M=64,DFF=2048; 8 cores x 2 batches each; no collectives.

  Math restructuring (validated against the jax reference):
   - rfft+mode-gather == x @ Fcat where Fcat[l, 0:64]=cos(2*pi*k_j*l/L),
     Fcat[l, 64:128]=-sin(...), k_j = mode_index.
   - Wq and Wo are applied in mode space (16x cheaper); k/v projections are
     dead code in the reference.
   - irfft of a spectrum with only bins 0..63 populated == P @ C2S2.
   - The Fourier branch contributes ~1e-4 absolute to an O(1) output, so the
     whole mode path (DFT input, DFT matrix, mode weights, mode data) runs in
     fp8e4m3; per-element ~6% error on a 1e-4 contribution is negligible.
   - series-decomp: the two moving averages are banded Toeplitz matmuls on the
     (otherwise idle) tensor engine, in token-major layout reached via the DMA
     XBAR transpose (SBUF->SBUF, bf16).  Output chunks of 96 tokens come from
     overlapping 128-token input windows (stride 96), so one stationary matrix
     per window size serves all interior chunks and replicate-padding folds
     into dedicated first/last-chunk stationaries.  The K=2 softmax gate is a
     sigmoid of weight/bias deltas; combines run token-major on Vector reading
     the PSUM means directly; the smooth mean M transposes back (bf16) and the
     final residual r = u - M keeps the carried stream in fp32.
   - FFN entirely in fp8e4 with DoubleRow matmuls; host prescales W1/W2.  The
     psum drain is a single vector scalar_tensor_tensor reading PSUM and
     adding the fp32 residual in place (no scalar copy / gpsimd add).
   - x reaches the device once as bf16 (x + bo folded on host): the iDFT
     residual add constructs the fp32 stream u = psy + xtb directly.

  Layout: device works feature-major ([D, L]) for the residual/FFN stream and
  token-major ([L-chunk, D]) for the decomposition means.
"""

import numpy as np

B, L, D, H, M, DFF = 16, 2048, 512, 8, 64, 2048
E = D // H
NC_ = 8
BLOC = B // NC_          # batches per core
NDC = D // 128           # 4 feature tiles
NFF = DFF // 128         # 16 dff tiles
NLC = L // 128           # 16 token chunks of 128
NTC = L // 512           # 4 token chunks of 512

# decomposition chunking: 16 non-overlapping 128-token windows; each mean is
# up to 3 accumulating matmuls (prev-tail band, in-window band, next-head
# band) so every engine access sits at partition offset 0 (the HW rejects
# wide accesses at nonzero partition offsets).
NW = 16

_prog_cache = {}
_fixn = [0]


def _fix_sync_waits(nc, max_waits=1, max_updates=4):
    """Split >max sem-waits/updates per instruction onto adjacent nops.

    The AWS neuronx-cc walrus rejects instructions carrying too many sync
    commands; Tile's tail drain aggregates one wait per outstanding semaphore.
    Engine-order execution makes the split semantically identical.
    """
    import concourse.mybir as mybir

    for f in nc.m.functions:
        for bb in f.blocks:
            insts = bb.instructions
            i = 0
            while i < len(insts):
                ins = insts[i]
                si = ins.sync_info
                if si is not None and si.on_wait and len(si.on_wait) > max_waits:
                    waits = list(si.on_wait)
                    si.on_wait = waits[-max_waits:]
                    rest = waits[:-max_waits]
                    chunks = [rest[j:j + max_waits]
                              for j in range(0, len(rest), max_waits)]
                    for c in reversed(chunks):
                        _fixn[0] += 1
                        nop = mybir.InstNoOp(name=f"I-fixw-{_fixn[0]}", ins=[], outs=[])
                        nop.engine = ins.engine
                        nop.sync_info = mybir.SyncInfo(on_wait=c, on_update=[])
                        insts.insert(i, nop)
                        i += 1
                if si is not None and si.on_update and len(si.on_update) > max_updates:
                    ups = list(si.on_update)
                    si.on_update = ups[:max_updates]
                    rest = ups[max_updates:]
                    chunks = [rest[j:j + max_updates]
                              for j in range(0, len(rest), max_updates)]
                    for c in chunks:
                        _fixn[0] += 1
                        nop = mybir.InstNoOp(name=f"I-fixu-{_fixn[0]}", ins=[], outs=[])
                        nop.engine = ins.engine
                        nop.sync_info = mybir.SyncInfo(on_wait=[], on_update=c)
                        insts.insert(i + 1, nop)
                        i += 1
                i += 1


def _build_program(need_bq, j0, w2scale, w1scale, w8scale, fix=True):
    import concourse.bass as bass
    import concourse.mybir as mybir
    from concourse.tile import TileContext

    F32 = mybir.dt.float32
    BF16 = mybir.dt.bfloat16
    FP8 = mybir.dt.float8e4
    AF = mybir.ActivationFunctionType
    OP = mybir.AluOpType

    nc = bass.Bass()

    # ---- DRAM I/O ----
    XTB = nc.dram_tensor("XTB", [BLOC, D, L], BF16, kind="ExternalInput")
    XBF = nc.dram_tensor("XBF", [BLOC, 128, NLC * D], FP8, kind="ExternalInput")
    FCT = nc.dram_tensor("FCT", [128, NLC * 128], FP8, kind="ExternalInput")
    C2S2 = nc.dram_tensor("C2S2", [128, L], BF16, kind="ExternalInput")
    WQT = nc.dram_tensor("WQT", [D, D], BF16, kind="ExternalInput")
    WOT = nc.dram_tensor("WOT", [D, D], BF16, kind="ExternalInput")
    WPK = nc.dram_tensor("WPK", [H, 128, M * 128], FP8, kind="ExternalInput")
    W1T = nc.dram_tensor("W1T", [D, DFF], FP8, kind="ExternalInput")
    W2T = nc.dram_tensor("W2T", [DFF, D], FP8, kind="ExternalInput")
    EYE = nc.dram_tensor("EYE", [128, 128], BF16, kind="ExternalInput")
    BQ4 = nc.dram_tensor("BQ4", [128, NDC], F32, kind="ExternalInput")
    AMAT = nc.dram_tensor("AMAT", [128, 1280], BF16, kind="ExternalInput")
    DECS = nc.dram_tensor("DECS", [128, 4], F32, kind="ExternalInput")
    OUT_T = nc.dram_tensor("OUT_T", [BLOC, D, L], F32, kind="ExternalOutput")

    with TileContext(nc) as tc:
        # ---------- persistent pools (explicit LIFO close at the end) ------
        cst = tc.tile_pool(name="cst", bufs=1)
        cstp = cst.__enter__()
        dec = tc.tile_pool(name="dec", bufs=1)
        decp = dec.__enter__()
        dtok = tc.tile_pool(name="dtok", bufs=4)
        dtokp = dtok.__enter__()

        # DMA issue order: what the front needs first.
        fct = cstp.tile([128, NLC * 128], FP8, name="fct")
        nc.sync.dma_start(out=fct[:], in_=FCT[:])
        wqt = [cstp.tile([128, D], BF16, name=f"wqt{i}") for i in range(NDC)]
        wot = [cstp.tile([128, D], BF16, name=f"wot{i}") for i in range(NDC)]
        eye = cstp.tile([128, 128], BF16, name="eye")
        c2s2 = cstp.tile([128, L], BF16, name="c2s2")
        amat = cstp.tile([128, 1280], BF16, name="amat")
        decs = cstp.tile([128, 4], F32, name="decs")
        bq4 = None

        mt = [[cstp.tile([128, L], F32, name=f"m_{b}_{dc}") for dc in range(NDC)]
              for b in range(BLOC)]
        r18 = [[cstp.tile([128, 2, L], FP8, name=f"r18_{b}_{kp}") for kp in range(2)]
               for b in range(BLOC)]
        # decomposition working tiles (persistent; single-buffered)
        ubf2 = decp.tile([128, NDC, L], BF16, name="ubf2")
        ut2 = decp.tile([128, NDC, NW, 128], BF16, name="ut2")
        mta2 = decp.tile([128, NW, D], BF16, name="mta2")
        mfm2 = decp.tile([128, NW, NDC, 128], BF16, name="mfm2")

        # ---------- series decomposition via tensor-engine banded means ----
        def decomp_pe(b, msp, dw_col, db_col, want_r1):
            """mt[b][*] (fp32 [128, L]) -> series-decomp residual, in place.

            u -> bf16 copy -> one batched DMA-xbar blocked transpose into
            token-major 128-token windows -> m25 / (m13-m25) as banded
            matmuls -> token-major combines on V reading PSUM -> smooth mean
            transposed back (4-chunk groups) -> r = u - M on gpsimd (fp32
            stream untouched by the bf16 mean path).
            """
            # cast + transpose per half-L: the half-0 pieces only depend on
            # the first two t4 slices of mt (u-adds / FFN drains), so the
            # early mean chunks start while the producer is still finishing.
            # The cast runs on gpsimd: its queue is empty, so it fires the
            # moment the data is ready instead of waiting behind the FFN
            # gelus in the scalar engine's FIFO.
            for hf in range(2):
                for dc in range(NDC):
                    hs = slice(hf * 1024, (hf + 1) * 1024)
                    nc.gpsimd.tensor_copy(ubf2[:, dc, hs], mt[b][dc][:, hs])
                    nc.sync.dma_start(out=ut2[:, dc, 8 * hf:8 * hf + 8, :],
                                      in_=ubf2[:, dc, hs], transpose=True)
            for c in range(NW):
                v = 1 if c == 0 else (3 if c == NW - 1 else 2)
                seq = ([(0, c - 1)] if c > 0 else []) + [(v, c)] + \
                    ([(4, c + 1)] if c < NW - 1 else [])
                m25 = msp.tile([128, D], F32, name="m25", tag="m25")
                dlt = msp.tile([128, D], F32, name="dlt", tag="dlt")
                # each PSUM column region must finish its accumulation group
                # before the next region starts (interleaved groups corrupt)
                for ps, boff in ((m25, 0), (dlt, 5)):
                    for dc in range(NDC):
                        for i, (blk, wc) in enumerate(seq):
                            nc.tensor.matmul(
                                ps[:, dc * 128:(dc + 1) * 128],
                                amat[:, (boff + blk) * 128:(boff + blk + 1) * 128],
                                ut2[:, dc, wc, :], start=(i == 0),
                                stop=(i == len(seq) - 1))
                g = dtokp.tile([128, NDC, 128], BF16, name="g", tag="g")
                nc.scalar.activation(g[:], ut2[:, :, c, :], AF.Sigmoid,
                                     scale=decs[:, dw_col:dw_col + 1],
                                     bias=decs[:, db_col:db_col + 1])
                q = dtokp.tile([128, D], BF16, name="q", tag="q")
                nc.vector.tensor_tensor(q[:], g.rearrange("p a b -> p (a b)"),
                                        dlt[:], OP.mult)
                nc.vector.tensor_tensor(mta2[:, c, :], m25[:], q[:], OP.add)
            for gi in range(4):
                nc.sync.dma_start(
                    out=mfm2[:, 4 * gi:4 * gi + 4, :, :],
                    in_=mta2[:, 4 * gi:4 * gi + 4, :], transpose=True)
            # r = u - M, split per half-L and across V/G so the tail
            # pipelines with the group transposes above
            for dc in range(NDC):
                mtv = mt[b][dc].rearrange("p (c t) -> p c t", t=128)
                for hf in range(2):
                    sl = (slice(None), slice(8 * hf, 8 * hf + 8), slice(None))
                    eng = nc.vector if (dc + hf) % 2 == 0 else nc.gpsimd
                    eng.tensor_tensor(mtv[sl], mtv[sl],
                                      mfm2[:, 8 * hf:8 * hf + 8, dc, :],
                                      OP.subtract)
                    if not want_r1:
                        nc.sync.dma_start(
                            out=OUT_T[b, dc * 128:(dc + 1) * 128,
                                      hf * 1024:(hf + 1) * 1024],
                            in_=mt[b][dc][:, hf * 1024:(hf + 1) * 1024])
                if want_r1:
                    nc.scalar.activation(r18[b][dc // 2][:, dc % 2, :],
                                         mt[b][dc][:], AF.Copy)

        # ---------- FFN weights (issued early; used after the Fourier phase)
        ffnw = tc.tile_pool(name="ffnw", bufs=1)
        ffnwp = ffnw.__enter__()
        w1dr = [ffnwp.tile([128, 2, DFF], FP8, name=f"w1dr{i}")
                for i in range(2)]
        w2dr = [ffnwp.tile([128, 2, D], FP8, name=f"w2dr{i}")
                for i in range(NFF // 2)]

        # ---------- Fourier branch (fp8 mode path) ----------
        ph1s = tc.tile_pool(name="ph1s", bufs=1)
        ph1sp = ph1s.__enter__()
        pcat = [ph1sp.tile([128, D], BF16, name=f"pcat{b}") for b in range(BLOC)]

        msp_cm = tc.tile_pool(name="msp", bufs=2, space="PSUM")

        with tc.tile_pool(name="ph1a", bufs=2, space="PSUM") as frp, \
             tc.tile_pool(name="ph1t", bufs=1, space="PSUM") as frpt, \
             tc.tile_pool(name="wpkp", bufs=4) as wpkp, \
             tc.tile_pool(name="xbfp", bufs=2) as xbfp:
            # x token-major fp8, streamed in quarter-L chunks
            xbfq = {}
            for b in range(BLOC):
                for qc in range(4):
                    xbfq[(b, qc)] = xbfp.tile([128, 4 * D], FP8,
                                              name=f"xb{b}_{qc}", tag="xb")
            for b in range(BLOC):
                for qc in range(4):
                    nc.sync.dma_start(out=xbfq[(b, qc)][:],
                                      in_=XBF[b][:, qc * 4 * D:(qc + 1) * 4 * D])
            # PE p-state warmup: dead matmuls on fct while xbf streams in;
            # the activity window (~3.4us) promotes the array to 2.4 GHz
            # before the first real DFT matmul
            wm = frpt.tile([128, 128], F32, name="wm", tag="wm")
            for i in range(24):
                nc.tensor.matmul(wm[:], fct[:, 0:128], fct[:, 0:128],
                                 start=(i == 0), stop=(i == 23))
            # mode-mix weight stream: all quarters issued up front; the
            # 4-buffer pool self-clocks arrival against consumption
            wpk_tiles = []
            for h in range(H):
                for q in range(4):
                    wq = wpkp.tile([128, 16 * 128], FP8, name=f"wpk{h}_{q}",
                                   tag="wpk")
                    nc.sync.dma_start(out=wq[:],
                                      in_=WPK[h][:, q * 2048:(q + 1) * 2048])
                    wpk_tiles.append(wq)
            for i in range(NDC):
                nc.sync.dma_start(out=wqt[i][:], in_=WQT[i * 128:(i + 1) * 128, :])
            nc.sync.dma_start(out=eye[:], in_=EYE[:])
            nc.sync.dma_start(out=c2s2[:], in_=C2S2[:])
            for i in range(NDC):
                nc.sync.dma_start(out=wot[i][:], in_=WOT[i * 128:(i + 1) * 128, :])
            nc.sync.dma_start(out=decs[:], in_=DECS[:])
            nc.sync.dma_start(out=amat[:], in_=AMAT[:])
            if need_bq:
                bq4 = cstp.tile([128, NDC], F32, name="bq4")
                nc.sync.dma_start(out=bq4[:], in_=BQ4[:])

            qt = [[None] * NDC for _ in range(BLOC)]
            for b in range(BLOC):
                # DFT: psD[m-ext, d] = sum_lc fct_lc^T @ xbf_lc (16 matmuls)
                psd = frp.tile([128, D], F32, name="psD", tag="psD")
                for lc in range(NLC):
                    nc.tensor.matmul(
                        psd[:], fct[:, lc * 128:(lc + 1) * 128],
                        xbfq[(b, lc // 4)][:, (lc % 4) * D:(lc % 4 + 1) * D],
                        start=(lc == 0), stop=(lc == NLC - 1))
                xsn = ph1sp.tile([128, D], BF16, name=f"xsn{b}", tag="xsn")
                nc.scalar.copy(xsn[:], psd[:])
                # transpose to xselT[d, m-ext]
                xselT = ph1sp.tile([128, NDC * 128], BF16, name=f"xselT{b}",
                                   tag="xselT")
                for dc in range(NDC):
                    pst = frpt.tile([128, 128], BF16, name="psT", tag="psT")
                    nc.tensor.transpose(pst[:], xsn[:, dc * 128:(dc + 1) * 128],
                                        eye[:])
                    nc.scalar.copy(xselT[:, dc * 128:(dc + 1) * 128], pst[:])
                # q-projection in mode space: QT[dout, m-ext]
                for do in range(NDC):
                    qt[b][do] = ph1sp.tile([128, 128], BF16, name=f"qt{b}_{do}",
                                           tag=f"qt{b}_{do}")
                    ps = frp.tile([128, 128], F32, name="psQ", tag="psQ")
                    for dc in range(NDC):
                        nc.tensor.matmul(
                            ps[:], wqt[dc][:, do * 128:(do + 1) * 128],
                            xselT[:, dc * 128:(dc + 1) * 128],
                            start=(dc == 0), stop=(dc == NDC - 1))
                    if need_bq:
                        nc.vector.tensor_tensor(
                            ps[:, j0:j0 + 1], ps[:, j0:j0 + 1],
                            bq4[:, do:do + 1], OP.add)
                    nc.scalar.copy(qt[b][do][:], ps[:])

            # mode mix: per head, per mode, complex ExE channel mix (fp8,
            # no DoubleRow: full-column bf16/fp8 weights get FWL).
            # RH_h rows: 0:64 = Qre e-rows, 64:128 = Qim e-rows; col = 2m + b
            rh = [ph1sp.tile([128, 128], FP8, name=f"rh{h}", tag=f"rh{h}")
                  for h in range(H)]
            for h in range(H):
                src_do, r0 = h // 2, (h % 2) * 64
                for b in range(BLOC):
                    rhv = rh[h].rearrange("p (m t) -> p m t", t=2)
                    nc.scalar.copy(rhv[0:64, :, b], qt[b][src_do][r0:r0 + 64, 0:64])
                    nc.scalar.copy(rhv[64:128, :, b], qt[b][src_do][r0:r0 + 64, 64:128])
            otre = [[ph1sp.tile([128, M], BF16, name=f"otre{b}_{dc}",
                                tag=f"otre{b}{dc}")
                     for dc in range(NDC)] for b in range(BLOC)]
            otim = [[ph1sp.tile([128, M], BF16, name=f"otim{b}_{dc}",
                                tag=f"otim{b}{dc}")
                     for dc in range(NDC)] for b in range(BLOC)]
            for h in range(H):
                psm = frp.tile([128, 128], F32, name="psM", tag="psM")
                for q in range(4):
                    wpk_q = wpk_tiles[h * 4 + q]
                    for mq in range(16):
                        m = q * 16 + mq
                        nc.tensor.matmul(
                            psm[:, 2 * m:2 * m + 2],
                            wpk_q[:, mq * 128:(mq + 1) * 128],
                            rh[h][:, 2 * m:2 * m + 2],
                            start=True, stop=True)
                psv = psm.rearrange("p (m t) -> p m t", t=2)
                dc, r0 = h // 2, (h % 2) * 64
                for b in range(BLOC):
                    nc.scalar.activation(otre[b][dc][r0:r0 + 64, :],
                                         psv[0:64, :, b], AF.Copy,
                                         scale=1.0 / w8scale)
                    nc.scalar.activation(otim[b][dc][r0:r0 + 64, :],
                                         psv[64:128, :, b], AF.Copy,
                                         scale=1.0 / w8scale)

            # bulk loads queued behind the phase-1-critical WPK stream:
            # x (feature-major bf16, +bo folded) lands in the idle decomp
            # buffers (ubf2 for b0, mta2 for b1), then the FFN weights
            mta2f = mta2.rearrange("p a b -> p (a b)")
            for dc in range(NDC):
                nc.sync.dma_start(out=ubf2[:, dc, :],
                                  in_=XTB[0, dc * 128:(dc + 1) * 128, :])
            for dc in range(NDC):
                nc.sync.dma_start(out=mta2f[:, dc * L:(dc + 1) * L],
                                  in_=XTB[1, dc * 128:(dc + 1) * 128, :])
            for i in range(2):
                nc.sync.dma_start(out=w1dr[i][:, 0, :],
                                  in_=W1T[(2 * i) * 128:(2 * i + 1) * 128, :])
                nc.sync.dma_start(out=w1dr[i][:, 1, :],
                                  in_=W1T[(2 * i + 1) * 128:(2 * i + 2) * 128, :])
            for i in range(NFF // 2):
                nc.sync.dma_start(out=w2dr[i][:, 0, :],
                                  in_=W2T[(2 * i) * 128:(2 * i + 1) * 128, :])
                nc.sync.dma_start(out=w2dr[i][:, 1, :],
                                  in_=W2T[(2 * i + 1) * 128:(2 * i + 2) * 128, :])

        # ph1a PSUM closed; open the means PSUM pool (lives through the rest)
        msp = msp_cm.__enter__()

        with tc.tile_pool(name="pswo", bufs=1, space="PSUM") as pswo, \
             tc.tile_pool(name="psy", bufs=2, space="PSUM") as psyp:
            for b in range(BLOC):
                # Wo projection, transposed orientation: pcat[m-ext, dout]
                for ro, ot in ((0, otre[b]), (64, otim[b])):
                    psw = pswo.tile([M, D], F32, name="psW", tag="psW")
                    for dc in range(NDC):
                        nc.tensor.matmul(
                            psw[:], ot[dc][:], wot[dc][:],
                            start=(dc == 0), stop=(dc == NDC - 1))
                    nc.scalar.copy(pcat[b][ro:ro + 64, :], psw[:])
                # iDFT + u = x+bo + yW   (feature-major, fp32, writes mt)
                for dc in range(NDC):
                    for t4 in range(NTC):
                        psy = psyp.tile([128, 512], F32, name="psY", tag="psY")
                        nc.tensor.matmul(
                            psy[:], pcat[b][:, dc * 128:(dc + 1) * 128],
                            c2s2[:, t4 * 512:(t4 + 1) * 512],
                            start=True, stop=True)
                        if b == 0:
                            xsl = ubf2[:, dc, t4 * 512:(t4 + 1) * 512]
                        else:
                            xsl = mta2.rearrange("p a b -> p (a b)")[
                                :, dc * L + t4 * 512:dc * L + (t4 + 1) * 512]
                        nc.vector.tensor_tensor(
                            mt[b][dc][:, t4 * 512:(t4 + 1) * 512],
                            psy[:], xsl, OP.add)
            # issued after BOTH batches' u-adds: the decomp scratch buffers
            # double as the landing zone for x (ubf2: b0, mta2: b1), so the
            # b1 reads must precede every decomp write in program order
            decomp_pe(0, msp, 0, 1, True)

        ph1s.__exit__(None, None, None)

        # ---------- FFN (fp8e4 DoubleRow; host-prescaled W1/W2) ----------
        def ffn(b):
            for t4 in range(NTC):
                t0, t1 = t4 * 512, (t4 + 1) * 512
                gq2 = [gqp.tile([128, 2, 512], FP8, name=f"gq{fp}",
                                tag=f"gq{fp}") for fp in range(NFF // 2)]
                for fp in range(NFF // 2):
                    for k in range(2):
                        ff = 2 * fp + k
                        psh = pshp.tile([128, 512], F32, name="psH", tag="psH")
                        for kp in range(2):
                            nc.tensor.matmul(
                                psh[:],
                                w1dr[kp][:, :, ff * 128:(ff + 1) * 128],
                                r18[b][kp][:, :, t0:t1],
                                start=(kp == 0), stop=(kp == 1),
                                perf_mode=mybir.MatmulPerfMode.DoubleRow)
                        nc.scalar.activation(gq2[fp][:, k, :], psh[:], AF.Gelu,
                                             scale=1.0 / w1scale)
                for do in range(NDC):
                    psf = psfp.tile([128, 512], F32, name="psF", tag="psF")
                    for fp in range(NFF // 2):
                        nc.tensor.matmul(
                            psf[:],
                            w2dr[fp][:, :, do * 128:(do + 1) * 128],
                            gq2[fp][:, :, :],
                            start=(fp == 0), stop=(fp == NFF // 2 - 1),
                            perf_mode=mybir.MatmulPerfMode.DoubleRow)
                    # drain + residual add in one vector op reading PSUM
                    sl = mt[b][do][:, t0:t1]
                    nc.vector.scalar_tensor_tensor(
                        sl, psf[:], 1.0 / w2scale, sl, OP.mult, OP.add)

        with tc.tile_pool(name="gqp", bufs=1) as gqp, \
             tc.tile_pool(name="pshp", bufs=2, space="PSUM") as pshp, \
             tc.tile_pool(name="psfp", bufs=2, space="PSUM") as psfp:
            decomp_pe(1, msp, 0, 1, True)
            ffn(0)
            decomp_pe(0, msp, 2, 3, False)
            ffn(1)
            decomp_pe(1, msp, 2, 3, False)

        ffnw.__exit__(None, None, None)
        msp_cm.__exit__(None, None, None)
        dtok.__exit__(None, None, None)
        dec.__exit__(None, None, None)
        cst.__exit__(None, None, None)

    if fix:
        _fix_sync_waits(nc)
    return nc


def _build_amat():
    """[128, 1280] float64: 10 [128,128] blocks
    [A25_m | A25_first | A25_int | A25_last | A25_p | B_m | B_first | B_int |
    B_last | B_p].  mean(c) = A_m^T u[c-1] + A_0^T u[c] + A_p^T u[c+1] with
    replicate padding folded into the first/last in-window variants.
    B = A13 - A25 so the delta matmul yields m13 - m25 directly."""
    def build(k):
        p = (k - 1) // 2
        A_m = np.zeros((128, 128))
        A_p = np.zeros((128, 128))
        A_f = np.zeros((128, 128))
        A_i = np.zeros((128, 128))
        A_l = np.zeros((128, 128))
        for j in range(128):
            for d in range(-p, p + 1):
                t = j + d
                if t < 0:
                    A_m[128 + t, j] += 1.0 / k
                    A_f[0, j] += 1.0 / k
                elif t > 127:
                    A_p[t - 128, j] += 1.0 / k
                    A_l[127, j] += 1.0 / k
                if 0 <= t <= 127:
                    A_f[t, j] += 1.0 / k
                    A_i[t, j] += 1.0 / k
                    A_l[t, j] += 1.0 / k
        return [A_m, A_f, A_i, A_l, A_p]
    A13 = build(13)
    A25 = build(25)
    return np.concatenate(A25 + [a13 - a25 for a13, a25 in zip(A13, A25)],
                          axis=1)


def _host_prep(inputs):
    import ml_dtypes
    bf16 = ml_dtypes.bfloat16
    fp8 = ml_dtypes.float8_e4m3
    x = np.asarray(inputs["x"], np.float32)
    w2t_pre = np.asarray(inputs["conv2_w"], np.float32).T
    w2scale = float(2.0 ** np.floor(np.log2(224.0 / np.abs(w2t_pre).max())))
    w1t_pre = np.asarray(inputs["conv1_w"], np.float32).T
    w1scale = float(2.0 ** np.floor(np.log2(224.0 / np.abs(w1t_pre).max())))
    modes = np.asarray(inputs["mode_index"]).astype(np.int64)
    l = np.arange(L, dtype=np.float64)
    ang = 2.0 * np.pi * np.outer(l, modes.astype(np.float64)) / L
    FC = np.concatenate([np.cos(ang), -np.sin(ang)], axis=1)          # [L, 128]
    m_out = np.arange(M, dtype=np.float64)
    w = np.where(m_out == 0, 1.0, 2.0) / L
    ang2 = 2.0 * np.pi * np.outer(m_out, l) / L
    C2 = np.concatenate([w[:, None] * np.cos(ang2),
                         w[:, None] * -np.sin(ang2)], axis=0)         # [128, L]

    FCT = FC.reshape(NLC, 128, 128).transpose(1, 0, 2).reshape(128, NLC * 128)

    wr = np.asarray(inputs["four_wr"], np.float64)   # [H, E, O, M]
    wi = np.asarray(inputs["four_wi"], np.float64)
    wpk = np.zeros((H, M, 128, 128), np.float64)
    wpk[:, :, 0:64, 0:64] = wr.transpose(0, 3, 1, 2)
    wpk[:, :, 0:64, 64:128] = wi.transpose(0, 3, 1, 2)
    wpk[:, :, 64:128, 0:64] = -wi.transpose(0, 3, 1, 2)
    wpk[:, :, 64:128, 64:128] = wr.transpose(0, 3, 1, 2)
    wmax = np.abs(wpk).max()
    w8scale = float(2.0 ** np.floor(np.log2(224.0 / max(wmax, 1e-30))))
    WPKh = (wpk * w8scale).transpose(0, 2, 1, 3).reshape(H, 128, M * 128)

    dec1_w = np.asarray(inputs["dec1_w"], np.float64)
    dec1_b = np.asarray(inputs["dec1_b"], np.float64)
    dec2_w = np.asarray(inputs["dec2_w"], np.float64)
    dec2_b = np.asarray(inputs["dec2_b"], np.float64)
    decs = np.zeros((128, 4), np.float32)
    decs[:, 0] = dec1_w[0] - dec1_w[1]
    decs[:, 1] = dec1_b[0] - dec1_b[1]
    decs[:, 2] = dec2_w[0] - dec2_w[1]
    decs[:, 3] = dec2_b[0] - dec2_b[1]

    bo = np.asarray(inputs["bo"], np.float32)
    bq = np.asarray(inputs["bq"], np.float32)
    zero_pos = np.nonzero(modes == 0)[0]
    need_bq = bool(len(zero_pos)) and bool(np.any(bq != 0))
    j0 = int(zero_pos[0]) if need_bq else 0
    BQ4 = np.ascontiguousarray((L * bq).reshape(NDC, 128).T).astype(np.float32)

    shared = {
        "FCT": FCT.astype(fp8),
        "C2S2": C2.astype(bf16),
        "WQT": np.ascontiguousarray(np.asarray(inputs["Wq"], np.float32).T).astype(bf16),
        "WOT": np.ascontiguousarray(np.asarray(inputs["Wo"], np.float32).T).astype(bf16),
        "WPK": WPKh.astype(fp8),
        "W1T": np.ascontiguousarray(w1t_pre * w1scale).astype(fp8),
        "W2T": np.ascontiguousarray(w2t_pre * w2scale).astype(fp8),
        "EYE": np.eye(128, dtype=np.float32).astype(bf16),
        "BQ4": BQ4,
        "AMAT": _build_amat().astype(bf16),
        "DECS": decs,
    }
    in_maps = []
    for c in range(NC_):
        xl = x[c * BLOC:(c + 1) * BLOC]                       # [2, L, D]
        xpb = xl + bo[None, None, :]
        XTBc = np.ascontiguousarray(xpb.transpose(0, 2, 1)).astype(bf16)
        xbf = xl.astype(fp8)                                  # [2, L, D]
        XBFc = np.ascontiguousarray(
            xbf.reshape(BLOC, NLC, 128, D).transpose(0, 2, 1, 3)
        ).reshape(BLOC, 128, NLC * D)
        im = dict(shared)
        im["XTB"] = XTBc
        im["XBF"] = XBFc
        in_maps.append(im)
    return in_maps, need_bq, j0, w2scale, w1scale, w8scale


def kernel(**inputs):
    from concourse.bass_utils import run_bass_kernel_spmd

    in_maps, need_bq, j0, w2scale, w1scale, w8scale = _host_prep(inputs)
    key = (need_bq, j0, w2scale, w1scale, w8scale)
    if key not in _prog_cache:
        _prog_cache[key] = _build_program(need_bq, j0, w2scale, w1scale, w8scale)
    nc = _prog_cache[key]
    res = run_bass_kernel_spmd(nc, in_maps, core_ids=list(range(NC_)))
    outs = []
    for c in range(NC_):
        ot = np.asarray(res.results[c]["OUT_T"])              # [2, D, L]
        outs.append(np.ascontiguousarray(ot.transpose(0, 2, 1)))
    return np.concatenate(outs, axis=0).astype(np.float32)


# revision 48
# speedup vs baseline: 1.1449x; 1.1080x over previous
"""FEDformer encoder layer on 8 TRN2 NeuronCores — batch-data-parallel Bass kernel.

Strategy (self-contained; shapes hardcoded):
  B=16,L=2048,D=512,H=8,E=64,M=64,DFF=2048; 8 cores x 2 batches each; no collectives.

  Math restructuring (validated against the jax reference):
   - rfft+mode-gather == x @ Fcat where Fcat[l, 0:64]=cos(2*pi*k_j*l/L),
     Fcat[l, 64:128]=-sin(...), k_j = mode_index.
   - Wq and Wo are applied in mode space (16x cheaper); k/v projections are
     dead code in the reference.
   - irfft of a spectrum with only bins 0..63 populated == P @ C2S2.
   - The Fourier branch contributes ~1e-4 absolute to an O(1) output, so the
     whole mode path (DFT input, DFT matrix, mode weights, mode data) runs in
     fp8e4m3; per-element ~6% error on a 1e-4 contribution is negligible.
   - series-decomp: the two moving averages are banded Toeplitz matmuls on the
     (otherwise idle) tensor engine, in token-major layout reached via the DMA
     XBAR transpose (SBUF->SBUF, bf16).  Output chunks of 96 tokens come from
     overlapping 128-token input windows (stride 96), so one stationary matrix
     per window size serves all interior chunks and replicate-padding folds
     into dedicated first/last-chunk stationaries.  The K=2 softmax gate is a
     sigmoid of weight/bias deltas; combines run token-major on Vector reading
     the PSUM means directly; the smooth mean M transposes back (bf16) and the
     final residual r = u - M keeps the carried stream in fp32.
   - FFN entirely in fp8e4 with DoubleRow matmuls; host prescales W1/W2.  The
     psum drain is a single vector scalar_tensor_tensor reading PSUM and
     adding the fp32 residual in place (no scalar copy / gpsimd add).
   - x reaches the device once as bf16 (x + bo folded on host): the iDFT
     residual add constructs the fp32 stream u = psy + xtb directly.

  Layout: device works feature-major ([D, L]) for the residual/FFN stream and
  token-major ([L-chunk, D]) for the decomposition means.
"""

import numpy as np

B, L, D, H, M, DFF = 16, 2048, 512, 8, 64, 2048
E = D // H
NC_ = 8
BLOC = B // NC_          # batches per core
NDC = D // 128           # 4 feature tiles
NFF = DFF // 128         # 16 dff tiles
NLC = L // 128           # 16 token chunks of 128
NTC = L // 512           # 4 token chunks of 512

# decomposition chunking: 16 non-overlapping 128-token windows; each mean is
# up to 3 accumulating matmuls (prev-tail band, in-window band, next-head
# band) so every engine access sits at partition offset 0 (the HW rejects
# wide accesses at nonzero partition offsets).
NW = 16

_prog_cache = {}
_fixn = [0]


def _fix_sync_waits(nc, max_waits=1, max_updates=4):
    """Split >max sem-waits/updates per instruction onto adjacent nops.

    The AWS neuronx-cc walrus rejects instructions carrying too many sync
    commands; Tile's tail drain aggregates one wait per outstanding semaphore.
    Engine-order execution makes the split semantically identical.
    """
    import concourse.mybir as mybir

    for f in nc.m.functions:
        for bb in f.blocks:
            insts = bb.instructions
            i = 0
            while i < len(insts):
                ins = insts[i]
                si = ins.sync_info
                if si is not None and si.on_wait and len(si.on_wait) > max_waits:
                    waits = list(si.on_wait)
                    si.on_wait = waits[-max_waits:]
                    rest = waits[:-max_waits]
                    chunks = [rest[j:j + max_waits]
                              for j in range(0, len(rest), max_waits)]
                    for c in reversed(chunks):
                        _fixn[0] += 1
                        nop = mybir.InstNoOp(name=f"I-fixw-{_fixn[0]}", ins=[], outs=[])
                        nop.engine = ins.engine
                        nop.sync_info = mybir.SyncInfo(on_wait=c, on_update=[])
                        insts.insert(i, nop)
                        i += 1
                if si is not None and si.on_update and len(si.on_update) > max_updates:
                    ups = list(si.on_update)
                    si.on_update = ups[:max_updates]
                    rest = ups[max_updates:]
                    chunks = [rest[j:j + max_updates]
                              for j in range(0, len(rest), max_updates)]
                    for c in chunks:
                        _fixn[0] += 1
                        nop = mybir.InstNoOp(name=f"I-fixu-{_fixn[0]}", ins=[], outs=[])
                        nop.engine = ins.engine
                        nop.sync_info = mybir.SyncInfo(on_wait=[], on_update=c)
                        insts.insert(i + 1, nop)
                        i += 1
                i += 1


def _build_program(need_bq, j0, w2scale, w1scale, w8scale, fix=True):
    import concourse.bass as bass
    import concourse.mybir as mybir
    from concourse.tile import TileContext

    F32 = mybir.dt.float32
    BF16 = mybir.dt.bfloat16
    FP8 = mybir.dt.float8e4
    AF = mybir.ActivationFunctionType
    OP = mybir.AluOpType

    nc = bass.Bass()

    # ---- DRAM I/O ----
    XTB = nc.dram_tensor("XTB", [BLOC, D, L], BF16, kind="ExternalInput")
    XBF = nc.dram_tensor("XBF", [BLOC, 128, NLC * D], FP8, kind="ExternalInput")
    FCT = nc.dram_tensor("FCT", [128, NLC * 128], FP8, kind="ExternalInput")
    C2S2 = nc.dram_tensor("C2S2", [128, L], BF16, kind="ExternalInput")
    WQT = nc.dram_tensor("WQT", [D, D], BF16, kind="ExternalInput")
    WOT = nc.dram_tensor("WOT", [D, D], BF16, kind="ExternalInput")
    WPK = nc.dram_tensor("WPK", [H, 128, M * 128], FP8, kind="ExternalInput")
    W1T = nc.dram_tensor("W1T", [D, DFF], FP8, kind="ExternalInput")
    W2T = nc.dram_tensor("W2T", [DFF, D], FP8, kind="ExternalInput")
    EYE = nc.dram_tensor("EYE", [128, 128], BF16, kind="ExternalInput")
    BQ4 = nc.dram_tensor("BQ4", [128, NDC], F32, kind="ExternalInput")
    AMAT = nc.dram_tensor("AMAT", [128, 1280], BF16, kind="ExternalInput")
    DECS = nc.dram_tensor("DECS", [128, 4], F32, kind="ExternalInput")
    OUT_T = nc.dram_tensor("OUT_T", [BLOC, D, L], F32, kind="ExternalOutput")

    with TileContext(nc) as tc:
        # ---------- persistent pools (explicit LIFO close at the end) ------
        cst = tc.tile_pool(name="cst", bufs=1)
        cstp = cst.__enter__()
        dec = tc.tile_pool(name="dec", bufs=1)
        decp = dec.__enter__()
        dtok = tc.tile_pool(name="dtok", bufs=4)
        dtokp = dtok.__enter__()

        # DMA issue order: what the front needs first.
        fct = cstp.tile([128, NLC * 128], FP8, name="fct")
        nc.sync.dma_start(out=fct[:], in_=FCT[:])
        wqt = [cstp.tile([128, D], BF16, name=f"wqt{i}") for i in range(NDC)]
        wot = [cstp.tile([128, D], BF16, name=f"wot{i}") for i in range(NDC)]
        eye = cstp.tile([128, 128], BF16, name="eye")
        c2s2 = cstp.tile([128, L], BF16, name="c2s2")
        amat = cstp.tile([128, 1280], BF16, name="amat")
        decs = cstp.tile([128, 4], F32, name="decs")
        bq4 = None

        mt = [[cstp.tile([128, L], F32, name=f"m_{b}_{dc}") for dc in range(NDC)]
              for b in range(BLOC)]
        r18 = [[cstp.tile([128, 2, L], FP8, name=f"r18_{b}_{kp}") for kp in range(2)]
               for b in range(BLOC)]
        # decomposition working tiles (persistent; single-buffered)
        ubf2 = decp.tile([128, NDC, L], BF16, name="ubf2")
        ut2 = decp.tile([128, NDC, NW, 128], BF16, name="ut2")
        mta2 = decp.tile([128, NW, D], BF16, name="mta2")
        mfm2 = decp.tile([128, NW, NDC, 128], BF16, name="mfm2")

        # ---------- series decomposition via tensor-engine banded means ----
        def decomp_pe(b, msp, dw_col, db_col, want_r1):
            """mt[b][*] (fp32 [128, L]) -> series-decomp residual, in place.

            u -> bf16 copy -> one batched DMA-xbar blocked transpose into
            token-major 128-token windows -> m25 / (m13-m25) as banded
            matmuls -> token-major combines on V reading PSUM -> smooth mean
            transposed back (4-chunk groups) -> r = u - M on gpsimd (fp32
            stream untouched by the bf16 mean path).
            """
            # cast + transpose per half-L: the half-0 pieces only depend on
            # the first two t4 slices of mt (u-adds / FFN drains), so the
            # early mean chunks start while the producer is still finishing
            for hf in range(2):
                for dc in range(NDC):
                    hs = slice(hf * 1024, (hf + 1) * 1024)
                    nc.scalar.activation(ubf2[:, dc, hs], mt[b][dc][:, hs],
                                         AF.Copy)
                    nc.sync.dma_start(out=ut2[:, dc, 8 * hf:8 * hf + 8, :],
                                      in_=ubf2[:, dc, hs], transpose=True)
            for c in range(NW):
                v = 1 if c == 0 else (3 if c == NW - 1 else 2)
                seq = ([(0, c - 1)] if c > 0 else []) + [(v, c)] + \
                    ([(4, c + 1)] if c < NW - 1 else [])
                m25 = msp.tile([128, D], F32, name="m25", tag="m25")
                dlt = msp.tile([128, D], F32, name="dlt", tag="dlt")
                # each PSUM column region must finish its accumulation group
                # before the next region starts (interleaved groups corrupt)
                for ps, boff in ((m25, 0), (dlt, 5)):
                    for dc in range(NDC):
                        for i, (blk, wc) in enumerate(seq):
                            nc.tensor.matmul(
                                ps[:, dc * 128:(dc + 1) * 128],
                                amat[:, (boff + blk) * 128:(boff + blk + 1) * 128],
                                ut2[:, dc, wc, :], start=(i == 0),
                                stop=(i == len(seq) - 1))
                g = dtokp.tile([128, NDC, 128], BF16, name="g", tag="g")
                nc.scalar.activation(g[:], ut2[:, :, c, :], AF.Sigmoid,
                                     scale=decs[:, dw_col:dw_col + 1],
                                     bias=decs[:, db_col:db_col + 1])
                q = dtokp.tile([128, D], BF16, name="q", tag="q")
                nc.vector.tensor_tensor(q[:], g.rearrange("p a b -> p (a b)"),
                                        dlt[:], OP.mult)
                nc.vector.tensor_tensor(mta2[:, c, :], m25[:], q[:], OP.add)
            for gi in range(4):
                nc.sync.dma_start(
                    out=mfm2[:, 4 * gi:4 * gi + 4, :, :],
                    in_=mta2[:, 4 * gi:4 * gi + 4, :], transpose=True)
            # r = u - M, split per half-L and across V/G so the tail
            # pipelines with the group transposes above
            for dc in range(NDC):
                mtv = mt[b][dc].rearrange("p (c t) -> p c t", t=128)
                for hf in range(2):
                    sl = (slice(None), slice(8 * hf, 8 * hf + 8), slice(None))
                    eng = nc.vector if (dc + hf) % 2 == 0 else nc.gpsimd
                    eng.tensor_tensor(mtv[sl], mtv[sl],
                                      mfm2[:, 8 * hf:8 * hf + 8, dc, :],
                                      OP.subtract)
                    if not want_r1:
                        nc.sync.dma_start(
                            out=OUT_T[b, dc * 128:(dc + 1) * 128,
                                      hf * 1024:(hf + 1) * 1024],
                            in_=mt[b][dc][:, hf * 1024:(hf + 1) * 1024])
                if want_r1:
                    nc.scalar.activation(r18[b][dc // 2][:, dc % 2, :],
                                         mt[b][dc][:], AF.Copy)

        # ---------- FFN weights (issued early; used after the Fourier phase)
        ffnw = tc.tile_pool(name="ffnw", bufs=1)
        ffnwp = ffnw.__enter__()
        w1dr = [ffnwp.tile([128, 2, DFF], FP8, name=f"w1dr{i}")
                for i in range(2)]
        w2dr = [ffnwp.tile([128, 2, D], FP8, name=f"w2dr{i}")
                for i in range(NFF // 2)]

        # ---------- Fourier branch (fp8 mode path) ----------
        ph1s = tc.tile_pool(name="ph1s", bufs=1)
        ph1sp = ph1s.__enter__()
        pcat = [ph1sp.tile([128, D], BF16, name=f"pcat{b}") for b in range(BLOC)]

        msp_cm = tc.tile_pool(name="msp", bufs=2, space="PSUM")

        with tc.tile_pool(name="ph1a", bufs=2, space="PSUM") as frp, \
             tc.tile_pool(name="ph1t", bufs=1, space="PSUM") as frpt, \
             tc.tile_pool(name="wpkp", bufs=4) as wpkp, \
             tc.tile_pool(name="xbfp", bufs=2) as xbfp:
            # x token-major fp8, streamed in quarter-L chunks
            xbfq = {}
            for b in range(BLOC):
                for qc in range(4):
                    xbfq[(b, qc)] = xbfp.tile([128, 4 * D], FP8,
                                              name=f"xb{b}_{qc}", tag="xb")
            for b in range(BLOC):
                for qc in range(4):
                    nc.sync.dma_start(out=xbfq[(b, qc)][:],
                                      in_=XBF[b][:, qc * 4 * D:(qc + 1) * 4 * D])
            # PE p-state warmup: dead matmuls on fct while xbf streams in;
            # the activity window (~3.4us) promotes the array to 2.4 GHz
            # before the first real DFT matmul
            wm = frpt.tile([128, 128], F32, name="wm", tag="wm")
            for i in range(24):
                nc.tensor.matmul(wm[:], fct[:, 0:128], fct[:, 0:128],
                                 start=(i == 0), stop=(i == 23))
            # mode-mix weight stream: all quarters issued up front; the
            # 4-buffer pool self-clocks arrival against consumption
            wpk_tiles = []
            for h in range(H):
                for q in range(4):
                    wq = wpkp.tile([128, 16 * 128], FP8, name=f"wpk{h}_{q}",
                                   tag="wpk")
                    nc.sync.dma_start(out=wq[:],
                                      in_=WPK[h][:, q * 2048:(q + 1) * 2048])
                    wpk_tiles.append(wq)
            for i in range(NDC):
                nc.sync.dma_start(out=wqt[i][:], in_=WQT[i * 128:(i + 1) * 128, :])
            nc.sync.dma_start(out=eye[:], in_=EYE[:])
            nc.sync.dma_start(out=c2s2[:], in_=C2S2[:])
            for i in range(NDC):
                nc.sync.dma_start(out=wot[i][:], in_=WOT[i * 128:(i + 1) * 128, :])
            nc.sync.dma_start(out=decs[:], in_=DECS[:])
            nc.sync.dma_start(out=amat[:], in_=AMAT[:])
            if need_bq:
                bq4 = cstp.tile([128, NDC], F32, name="bq4")
                nc.sync.dma_start(out=bq4[:], in_=BQ4[:])

            qt = [[None] * NDC for _ in range(BLOC)]
            for b in range(BLOC):
                # DFT: psD[m-ext, d] = sum_lc fct_lc^T @ xbf_lc (16 matmuls)
                psd = frp.tile([128, D], F32, name="psD", tag="psD")
                for lc in range(NLC):
                    nc.tensor.matmul(
                        psd[:], fct[:, lc * 128:(lc + 1) * 128],
                        xbfq[(b, lc // 4)][:, (lc % 4) * D:(lc % 4 + 1) * D],
                        start=(lc == 0), stop=(lc == NLC - 1))
                xsn = ph1sp.tile([128, D], BF16, name=f"xsn{b}", tag="xsn")
                nc.scalar.copy(xsn[:], psd[:])
                # transpose to xselT[d, m-ext]
                xselT = ph1sp.tile([128, NDC * 128], BF16, name=f"xselT{b}",
                                   tag="xselT")
                for dc in range(NDC):
                    pst = frpt.tile([128, 128], BF16, name="psT", tag="psT")
                    nc.tensor.transpose(pst[:], xsn[:, dc * 128:(dc + 1) * 128],
                                        eye[:])
                    nc.scalar.copy(xselT[:, dc * 128:(dc + 1) * 128], pst[:])
                # q-projection in mode space: QT[dout, m-ext]
                for do in range(NDC):
                    qt[b][do] = ph1sp.tile([128, 128], BF16, name=f"qt{b}_{do}",
                                           tag=f"qt{b}_{do}")
                    ps = frp.tile([128, 128], F32, name="psQ", tag="psQ")
                    for dc in range(NDC):
                        nc.tensor.matmul(
                            ps[:], wqt[dc][:, do * 128:(do + 1) * 128],
                            xselT[:, dc * 128:(dc + 1) * 128],
                            start=(dc == 0), stop=(dc == NDC - 1))
                    if need_bq:
                        nc.vector.tensor_tensor(
                            ps[:, j0:j0 + 1], ps[:, j0:j0 + 1],
                            bq4[:, do:do + 1], OP.add)
                    nc.scalar.copy(qt[b][do][:], ps[:])

            # mode mix: per head, per mode, complex ExE channel mix (fp8,
            # no DoubleRow: full-column bf16/fp8 weights get FWL).
            # RH_h rows: 0:64 = Qre e-rows, 64:128 = Qim e-rows; col = 2m + b
            rh = [ph1sp.tile([128, 128], FP8, name=f"rh{h}", tag=f"rh{h}")
                  for h in range(H)]
            for h in range(H):
                src_do, r0 = h // 2, (h % 2) * 64
                for b in range(BLOC):
                    rhv = rh[h].rearrange("p (m t) -> p m t", t=2)
                    nc.scalar.copy(rhv[0:64, :, b], qt[b][src_do][r0:r0 + 64, 0:64])
                    nc.scalar.copy(rhv[64:128, :, b], qt[b][src_do][r0:r0 + 64, 64:128])
            otre = [[ph1sp.tile([128, M], BF16, name=f"otre{b}_{dc}",
                                tag=f"otre{b}{dc}")
                     for dc in range(NDC)] for b in range(BLOC)]
            otim = [[ph1sp.tile([128, M], BF16, name=f"otim{b}_{dc}",
                                tag=f"otim{b}{dc}")
                     for dc in range(NDC)] for b in range(BLOC)]
            for h in range(H):
                psm = frp.tile([128, 128], F32, name="psM", tag="psM")
                for q in range(4):
                    wpk_q = wpk_tiles[h * 4 + q]
                    for mq in range(16):
                        m = q * 16 + mq
                        nc.tensor.matmul(
                            psm[:, 2 * m:2 * m + 2],
                            wpk_q[:, mq * 128:(mq + 1) * 128],
                            rh[h][:, 2 * m:2 * m + 2],
                            start=True, stop=True)
                psv = psm.rearrange("p (m t) -> p m t", t=2)
                dc, r0 = h // 2, (h % 2) * 64
                for b in range(BLOC):
                    nc.scalar.activation(otre[b][dc][r0:r0 + 64, :],
                                         psv[0:64, :, b], AF.Copy,
                                         scale=1.0 / w8scale)
                    nc.scalar.activation(otim[b][dc][r0:r0 + 64, :],
                                         psv[64:128, :, b], AF.Copy,
                                         scale=1.0 / w8scale)

            # bulk loads queued behind the phase-1-critical WPK stream:
            # x (feature-major bf16, +bo folded) lands in the idle decomp
            # buffers (ubf2 for b0, mta2 for b1), then the FFN weights
            mta2f = mta2.rearrange("p a b -> p (a b)")
            for dc in range(NDC):
                nc.sync.dma_start(out=ubf2[:, dc, :],
                                  in_=XTB[0, dc * 128:(dc + 1) * 128, :])
            for dc in range(NDC):
                nc.sync.dma_start(out=mta2f[:, dc * L:(dc + 1) * L],
                                  in_=XTB[1, dc * 128:(dc + 1) * 128, :])
            for i in range(2):
                nc.sync.dma_start(out=w1dr[i][:, 0, :],
                                  in_=W1T[(2 * i) * 128:(2 * i + 1) * 128, :])
                nc.sync.dma_start(out=w1dr[i][:, 1, :],
                                  in_=W1T[(2 * i + 1) * 128:(2 * i + 2) * 128, :])
            for i in range(NFF // 2):
                nc.sync.dma_start(out=w2dr[i][:, 0, :],
                                  in_=W2T[(2 * i) * 128:(2 * i + 1) * 128, :])
                nc.sync.dma_start(out=w2dr[i][:, 1, :],
                                  in_=W2T[(2 * i + 1) * 128:(2 * i + 2) * 128, :])

        # ph1a PSUM closed; open the means PSUM pool (lives through the rest)
        msp = msp_cm.__enter__()

        with tc.tile_pool(name="pswo", bufs=1, space="PSUM") as pswo, \
             tc.tile_pool(name="psy", bufs=2, space="PSUM") as psyp:
            for b in range(BLOC):
                # Wo projection, transposed orientation: pcat[m-ext, dout]
                for ro, ot in ((0, otre[b]), (64, otim[b])):
                    psw = pswo.tile([M, D], F32, name="psW", tag="psW")
                    for dc in range(NDC):
                        nc.tensor.matmul(
                            psw[:], ot[dc][:], wot[dc][:],
                            start=(dc == 0), stop=(dc == NDC - 1))
                    nc.scalar.copy(pcat[b][ro:ro + 64, :], psw[:])
                # iDFT + u = x+bo + yW   (feature-major, fp32, writes mt)
                for dc in range(NDC):
                    for t4 in range(NTC):
                        psy = psyp.tile([128, 512], F32, name="psY", tag="psY")
                        nc.tensor.matmul(
                            psy[:], pcat[b][:, dc * 128:(dc + 1) * 128],
                            c2s2[:, t4 * 512:(t4 + 1) * 512],
                            start=True, stop=True)
                        if b == 0:
                            xsl = ubf2[:, dc, t4 * 512:(t4 + 1) * 512]
                        else:
                            xsl = mta2.rearrange("p a b -> p (a b)")[
                                :, dc * L + t4 * 512:dc * L + (t4 + 1) * 512]
                        nc.vector.tensor_tensor(
                            mt[b][dc][:, t4 * 512:(t4 + 1) * 512],
                            psy[:], xsl, OP.add)
            # issued after BOTH batches' u-adds: the decomp scratch buffers
            # double as the landing zone for x (ubf2: b0, mta2: b1), so the
            # b1 reads must precede every decomp write in program order
            decomp_pe(0, msp, 0, 1, True)

        ph1s.__exit__(None, None, None)

        # ---------- FFN (fp8e4 DoubleRow; host-prescaled W1/W2) ----------
        def ffn(b):
            for t4 in range(NTC):
                t0, t1 = t4 * 512, (t4 + 1) * 512
                gq2 = [gqp.tile([128, 2, 512], FP8, name=f"gq{fp}",
                                tag=f"gq{fp}") for fp in range(NFF // 2)]
                for fp in range(NFF // 2):
                    for k in range(2):
                        ff = 2 * fp + k
                        psh = pshp.tile([128, 512], F32, name="psH", tag="psH")
                        for kp in range(2):
                            nc.tensor.matmul(
                                psh[:],
                                w1dr[kp][:, :, ff * 128:(ff + 1) * 128],
                                r18[b][kp][:, :, t0:t1],
                                start=(kp == 0), stop=(kp == 1),
                                perf_mode=mybir.MatmulPerfMode.DoubleRow)
                        nc.scalar.activation(gq2[fp][:, k, :], psh[:], AF.Gelu,
                                             scale=1.0 / w1scale)
                for do in range(NDC):
                    psf = psfp.tile([128, 512], F32, name="psF", tag="psF")
                    for fp in range(NFF // 2):
                        nc.tensor.matmul(
                            psf[:],
                            w2dr[fp][:, :, do * 128:(do + 1) * 128],
                            gq2[fp][:, :, :],
                            start=(fp == 0), stop=(fp == NFF // 2 - 1),
                            perf_mode=mybir.MatmulPerfMode.DoubleRow)
                    # drain + residual add in one vector op reading PSUM
                    sl = mt[b][do][:, t0:t1]
                    nc.vector.scalar_tensor_tensor(
                        sl, psf[:], 1.0 / w2scale, sl, OP.mult, OP.add)

        with tc.tile_pool(name="gqp", bufs=1) as gqp, \
             tc.tile_pool(name="pshp", bufs=2, space="PSUM") as pshp, \
             tc.tile_pool(name="psfp", bufs=2, space="PSUM") as psfp:
            decomp_pe(1, msp, 0, 1, True)
            ffn(0)
            decomp_pe(0, msp, 2, 3, False)
            ffn(1)
            decomp_pe(1, msp, 2, 3, False)

        ffnw.__exit__(None, None, None)
        msp_cm.__exit__(None, None, None)
        dtok.__exit__(None, None, None)
        dec.__exit__(None, None, None)
        cst.__exit__(None, None, None)

    if fix:
        _fix_sync_waits(nc)
    return nc


def _build_amat():
    """[128, 1280] float64: 10 [128,128] blocks
    [A25_m | A25_first | A25_int | A25_last | A25_p | B_m | B_first | B_int |
    B_last | B_p].  mean(c) = A_m^T u[c-1] + A_0^T u[c] + A_p^T u[c+1] with
    replicate padding folded into the first/last in-window variants.
    B = A13 - A25 so the delta matmul yields m13 - m25 directly."""
    def build(k):
        p = (k - 1) // 2
        A_m = np.zeros((128, 128))
        A_p = np.zeros((128, 128))
        A_f = np.zeros((128, 128))
        A_i = np.zeros((128, 128))
        A_l = np.zeros((128, 128))
        for j in range(128):
            for d in range(-p, p + 1):
                t = j + d
                if t < 0:
                    A_m[128 + t, j] += 1.0 / k
                    A_f[0, j] += 1.0 / k
                elif t > 127:
                    A_p[t - 128, j] += 1.0 / k
                    A_l[127, j] += 1.0 / k
                if 0 <= t <= 127:
                    A_f[t, j] += 1.0 / k
                    A_i[t, j] += 1.0 / k
                    A_l[t, j] += 1.0 / k
        return [A_m, A_f, A_i, A_l, A_p]
    A13 = build(13)
    A25 = build(25)
    return np.concatenate(A25 + [a13 - a25 for a13, a25 in zip(A13, A25)],
                          axis=1)


def _host_prep(inputs):
    import ml_dtypes
    bf16 = ml_dtypes.bfloat16
    fp8 = ml_dtypes.float8_e4m3
    x = np.asarray(inputs["x"], np.float32)
    w2t_pre = np.asarray(inputs["conv2_w"], np.float32).T
    w2scale = float(2.0 ** np.floor(np.log2(224.0 / np.abs(w2t_pre).max())))
    w1t_pre = np.asarray(inputs["conv1_w"], np.float32).T
    w1scale = float(2.0 ** np.floor(np.log2(224.0 / np.abs(w1t_pre).max())))
    modes = np.asarray(inputs["mode_index"]).astype(np.int64)
    l = np.arange(L, dtype=np.float64)
    ang = 2.0 * np.pi * np.outer(l, modes.astype(np.float64)) / L
    FC = np.concatenate([np.cos(ang), -np.sin(ang)], axis=1)          # [L, 128]
    m_out = np.arange(M, dtype=np.float64)
    w = np.where(m_out == 0, 1.0, 2.0) / L
    ang2 = 2.0 * np.pi * np.outer(m_out, l) / L
    C2 = np.concatenate([w[:, None] * np.cos(ang2),
                         w[:, None] * -np.sin(ang2)], axis=0)         # [128, L]

    FCT = FC.reshape(NLC, 128, 128).transpose(1, 0, 2).reshape(128, NLC * 128)

    wr = np.asarray(inputs["four_wr"], np.float64)   # [H, E, O, M]
    wi = np.asarray(inputs["four_wi"], np.float64)
    wpk = np.zeros((H, M, 128, 128), np.float64)
    wpk[:, :, 0:64, 0:64] = wr.transpose(0, 3, 1, 2)
    wpk[:, :, 0:64, 64:128] = wi.transpose(0, 3, 1, 2)
    wpk[:, :, 64:128, 0:64] = -wi.transpose(0, 3, 1, 2)
    wpk[:, :, 64:128, 64:128] = wr.transpose(0, 3, 1, 2)
    wmax = np.abs(wpk).max()
    w8scale = float(2.0 ** np.floor(np.log2(224.0 / max(wmax, 1e-30))))
    WPKh = (wpk * w8scale).transpose(0, 2, 1, 3).reshape(H, 128, M * 128)

    dec1_w = np.asarray(inputs["dec1_w"], np.float64)
    dec1_b = np.asarray(inputs["dec1_b"], np.float64)
    dec2_w = np.asarray(inputs["dec2_w"], np.float64)
    dec2_b = np.asarray(inputs["dec2_b"], np.float64)
    decs = np.zeros((128, 4), np.float32)
    decs[:, 0] = dec1_w[0] - dec1_w[1]
    decs[:, 1] = dec1_b[0] - dec1_b[1]
    decs[:, 2] = dec2_w[0] - dec2_w[1]
    decs[:, 3] = dec2_b[0] - dec2_b[1]

    bo = np.asarray(inputs["bo"], np.float32)
    bq = np.asarray(inputs["bq"], np.float32)
    zero_pos = np.nonzero(modes == 0)[0]
    need_bq = bool(len(zero_pos)) and bool(np.any(bq != 0))
    j0 = int(zero_pos[0]) if need_bq else 0
    BQ4 = np.ascontiguousarray((L * bq).reshape(NDC, 128).T).astype(np.float32)

    shared = {
        "FCT": FCT.astype(fp8),
        "C2S2": C2.astype(bf16),
        "WQT": np.ascontiguousarray(np.asarray(inputs["Wq"], np.float32).T).astype(bf16),
        "WOT": np.ascontiguousarray(np.asarray(inputs["Wo"], np.float32).T).astype(bf16),
        "WPK": WPKh.astype(fp8),
        "W1T": np.ascontiguousarray(w1t_pre * w1scale).astype(fp8),
        "W2T": np.ascontiguousarray(w2t_pre * w2scale).astype(fp8),
        "EYE": np.eye(128, dtype=np.float32).astype(bf16),
        "BQ4": BQ4,
        "AMAT": _build_amat().astype(bf16),
        "DECS": decs,
    }
    in_maps = []
    for c in range(NC_):
        xl = x[c * BLOC:(c + 1) * BLOC]                       # [2, L, D]
        xpb = xl + bo[None, None, :]
        XTBc = np.ascontiguousarray(xpb.transpose(0, 2, 1)).astype(bf16)
        xbf = xl.astype(fp8)                                  # [2, L, D]
        XBFc = np.ascontiguousarray(
            xbf.reshape(BLOC, NLC, 128, D).transpose(0, 2, 1, 3)
        ).reshape(BLOC, 128, NLC * D)
        im = dict(shared)
        im["XTB"] = XTBc
        im["XBF"] = XBFc
        in_maps.append(im)
    return in_maps, need_bq, j0, w2scale, w1scale, w8scale


def kernel(**inputs):
    from concourse.bass_utils import run_bass_kernel_spmd

    in_maps, need_bq, j0, w2scale, w1scale, w8scale = _host_prep(inputs)
    key = (need_bq, j0, w2scale, w1scale, w8scale)
    if key not in _prog_cache:
        _prog_cache[key] = _build_program(need_bq, j0, w2scale, w1scale, w8scale)
    nc = _prog_cache[key]
    res = run_bass_kernel_spmd(nc, in_maps, core_ids=list(range(NC_)))
    outs = []
    for c in range(NC_):
        ot = np.asarray(res.results[c]["OUT_T"])              # [2, D, L]
        outs.append(np.ascontiguousarray(ot.transpose(0, 2, 1)))
    return np.concatenate(outs, axis=0).astype(np.float32)
